# revision 5
# baseline (speedup 1.0000x reference)
"""nn_Decoder — LSTM (B=32, T=256, V=H=1024) + dense head on 8 TRN2 NeuronCores.

Strategy: 8-way model parallelism over the 4H gate dimension.
  - Each core owns a 128-wide slice of the hidden state and the matching
    512 columns (4 gates x 128) of Wi/Wh, gate-permuted to [i|f|o|g].
  - A = x @ Wi + b is precomputed in [128 x 512] tiles (4 timesteps per tile)
    on the tensor engine, interleaved with the recurrence, staged per step and
    injected into the z PSUM accumulation via an identity matmul.
  - Per step: z = A_t + h @ Wh_slice accumulates in PSUM (fp32r matmuls),
    sigmoid/tanh on ScalarE, state update on VectorE, h-slice transposed and
    AllGathered so every core has the full h^T for the next step.
  - Gathered h^T for every step is stored to DRAM; the dense head
    x = hs @ Wd + bd runs V-sharded (each core computes a 128-wide vocab
    slice for all (b, t) rows as x^T tiles).
Host glue shards/permutes inputs per core and reassembles (c, h, x).
"""

import numpy as np
import ml_dtypes
import concourse.bass as bass
import concourse.bacc as bacc
import concourse.mybir as mybir
from concourse import tile

F32 = mybir.dt.float32
F32R = mybir.dt.float32r
BF16 = mybir.dt.bfloat16
AF = mybir.ActivationFunctionType

B, V, H, T = 32, 1024, 1024, 256
NC = 8
SH = 4 * H // NC          # 512 per-core z-slice
HS = H // NC              # 128 per-core h-slice
KC = 8                    # 128-wide K chunks in V / H
NROW = B * T // 512       # dense row groups
TPG = 512 // B            # timesteps per dense row group


N_WARM = 0


def build_kernel(a_ahead: int = 2):
    TG = T // 4
    nc = bacc.Bacc("TRN2", target_bir_lowering=False, debug=False, num_devices=NC)

    xT = nc.dram_tensor("xT", [V, T, B], F32R, kind="ExternalInput")
    Wi_s = nc.dram_tensor("Wi_s", [V, SH], F32R, kind="ExternalInput")
    Wh_s = nc.dram_tensor("Wh_s", [H, SH], BF16, kind="ExternalInput")
    b_s = nc.dram_tensor("b_s", [1, SH], F32R, kind="ExternalInput")
    Wd_s = nc.dram_tensor("Wd_s", [H, HS], BF16, kind="ExternalInput")
    bd_s = nc.dram_tensor("bd_s", [HS, 1], F32, kind="ExternalInput")
    c0_s = nc.dram_tensor("c0_s", [B, HS], F32, kind="ExternalInput")
    h0T = nc.dram_tensor("h0T", [128, NC * B], BF16, kind="ExternalInput")
    ident = nc.dram_tensor("ident", [32, 32], F32R, kind="ExternalInput")
    ones1 = nc.dram_tensor("ones1", [1, 128], F32R, kind="ExternalInput")

    c_out = nc.dram_tensor("c_out", [B, HS], F32, kind="ExternalOutput")
    h_out = nc.dram_tensor("h_out", [B, HS], F32, kind="ExternalOutput")
    xd_out = nc.dram_tensor("xd_out", [NROW, HS, 512], F32, kind="ExternalOutput")

    cc_in = [nc.dram_tensor(f"cc_in{p}", [128, B], BF16) for p in range(2)]
    cc_out = [
        nc.dram_tensor(f"cc_out{p}", [NC * 128, B], BF16, addr_space="Shared")
        for p in range(2)
    ]
    hsT_dram = nc.dram_tensor("hsT_dram", [T, 128, NC * B], BF16)

    with tile.TileContext(nc) as tc:
        with (
            tc.tile_pool(name="weights", bufs=1) as wpool,
            tc.tile_pool(name="state", bufs=1) as spool,
            tc.tile_pool(name="hbuf", bufs=3) as hpool,
            tc.tile_pool(name="xt", bufs=2 * KC + 2) as xtpool,
            tc.tile_pool(name="astash", bufs=3) as aspool,
            tc.tile_pool(name="astage", bufs=5) as astpool,
            tc.tile_pool(name="gates", bufs=2) as gpool,
            tc.tile_pool(name="tmp", bufs=2) as tpool,
            tc.tile_pool(name="hsnd", bufs=2) as hsndpool,
            tc.tile_pool(name="zpsum", bufs=2, space="PSUM") as zpool,
            tc.tile_pool(name="apsum", bufs=2, space="PSUM") as apool,
            tc.tile_pool(name="dpsum", bufs=2, space="PSUM") as dpool,
            tc.tile_pool(name="dense", bufs=2) as dnpool,
        ):
            wi_sb = wpool.tile([128, KC * SH], F32R)
            wh_sb = wpool.tile([128, KC * SH], BF16)
            for k in range(KC):
                nc.sync.dma_start(out=wi_sb[:, SH * k : SH * (k + 1)],
                                  in_=Wi_s[128 * k : 128 * (k + 1), :])
                nc.sync.dma_start(out=wh_sb[:, SH * k : SH * (k + 1)],
                                  in_=Wh_s[128 * k : 128 * (k + 1), :])
            b_sb = wpool.tile([1, SH], F32R)
            nc.sync.dma_start(out=b_sb[:], in_=b_s[:])
            id_sb = wpool.tile([32, 32], F32R)
            nc.sync.dma_start(out=id_sb[:], in_=ident[:])
            on_sb = wpool.tile([1, 128], F32R)
            nc.sync.dma_start(out=on_sb[:], in_=ones1[:])

            c_sb = spool.tile([B, HS], F32)
            nc.sync.dma_start(out=c_sb[:], in_=c0_s[:])
            hbuf = hpool.tile([128, NC * B], BF16)
            nc.sync.dma_start(out=hbuf[:], in_=h0T[:])

            def emit_a_group(jg):
                ap = apool.tile([128, SH], F32)
                nc.tensor.matmul(ap[:], lhsT=on_sb[:], rhs=b_sb[:],
                                 start=True, stop=False)
                for k in range(KC):
                    xt = xtpool.tile([128, 128], F32R, tag="xt")
                    nc.sync.dma_start(
                        out=xt[:],
                        in_=xT[128 * k : 128 * (k + 1), 4 * jg : 4 * jg + 4, :],
                    )
                    nc.tensor.matmul(ap[:], lhsT=xt[:],
                                     rhs=wi_sb[:, SH * k : SH * (k + 1)],
                                     start=False, stop=(k == KC - 1))
                st = aspool.tile([128, SH], F32R, tag="astash")
                nc.vector.tensor_copy(st[:], ap[:])
                return st

            stash = {}
            stage = {}

            def emit_a_stage(t, st):
                sg = astpool.tile([32, SH], F32R, tag="astage")
                ts = t % 4
                nc.sync.dma_start(out=sg[:], in_=st[32 * ts : 32 * ts + 32, :])
                return sg

            for jg in range(min(a_ahead + 1, TG)):
                stash[jg] = emit_a_group(jg)
            for t in range(min(3, T)):
                stage[t] = emit_a_stage(t, stash[0])

            for t in range(T):
                bi = t % 2
                jg = t // 4
                if t % 4 == 0:
                    ng = jg + a_ahead + 1
                    if ng < TG and ng not in stash:
                        stash[ng] = emit_a_group(ng)
                        stash.pop(ng - a_ahead - 2, None)
                for ts in (t + 2, t + 3):
                    if ts < T and ts not in stage and (ts // 4) in stash:
                        stage[ts] = emit_a_stage(ts, stash[ts // 4])

                # half 1: gates g, f   |   half 2: gates i, o
                z1 = zpool.tile([B, SH // 2], F32, tag="z1")
                z2 = zpool.tile([B, SH // 2], F32, tag="z2")
                nc.tensor.matmul(z1[:], lhsT=id_sb[:], rhs=stage[t][:, 0:256],
                                 start=True, stop=False)
                for j in range(KC):
                    nc.tensor.matmul(
                        z1[:],
                        lhsT=hbuf[:, 32 * j : 32 * j + 32],
                        rhs=wh_sb[:, SH * j : SH * j + 256],
                        start=False, stop=(j == KC - 1),
                    )
                nc.tensor.matmul(z2[:], lhsT=id_sb[:], rhs=stage[t][:, 256:512],
                                 start=True, stop=False)
                for j in range(KC):
                    nc.tensor.matmul(
                        z2[:],
                        lhsT=hbuf[:, 32 * j : 32 * j + 32],
                        rhs=wh_sb[:, SH * j + 256 : SH * (j + 1)],
                        start=False, stop=(j == KC - 1),
                    )
                stage.pop(t, None)

                g_t = gpool.tile([B, 128], F32, tag="g")
                nc.scalar.activation(g_t[:], z1[:, 0:128], AF.Tanh)
                sf = gpool.tile([B, 128], F32, tag="sf")
                nc.scalar.activation(sf[:], z1[:, 128:256], AF.Sigmoid)
                sio = gpool.tile([B, 256], F32, tag="sio")
                nc.scalar.activation(sio[:], z2[:], AF.Sigmoid)

                fc = tpool.tile([B, 128], F32, tag="fc")
                nc.vector.tensor_mul(fc[:], sf[:], c_sb[:])
                ig = tpool.tile([B, 128], F32, tag="ig")
                nc.vector.tensor_mul(ig[:], sio[:, 0:128], g_t[:])
                nc.vector.tensor_add(c_sb[:], fc[:], ig[:])
                tc_t = tpool.tile([B, 128], F32, tag="tc")
                nc.scalar.activation(tc_t[:], c_sb[:], AF.Tanh)
                h_sb = tpool.tile([B, 128], BF16, tag="h")
                nc.vector.tensor_mul(h_sb[:], sio[:, 128:256], tc_t[:])

                if t == T - 1:
                    nc.sync.dma_start(out=c_out[:], in_=c_sb[:])
                    hf = tpool.tile([B, 128], F32, tag="hf")
                    nc.vector.tensor_mul(hf[:], sio[:, 128:256], tc_t[:])
                    nc.sync.dma_start(out=h_out[:], in_=hf[:])

                hT = hsndpool.tile([128, B], BF16, tag="hT")
                nc.sync.dma_start_transpose(out=hT[:], in_=h_sb[:])

                snd_dma = nc.sync.dma_start(out=cc_in[bi][:], in_=hT[:])
                nc.gpsimd.collective_compute(
                    "AllGather",
                    mybir.AluOpType.bypass,
                    ins=[cc_in[bi][:]],
                    outs=[cc_out[bi][:]],
                    replica_groups=[list(range(NC))],
                )
                hbuf = hpool.tile([128, NC * B], BF16, tag="hbuf")
                nc.sync.dma_start(
                    out=hbuf[:],
                    in_=cc_out[bi].ap().rearrange("(j p) b -> p j b", p=128),
                )
                nc.sync.dma_start(out=hsT_dram[t], in_=hbuf[:])

            # dense head: x^T v-slice
            wd_sb = wpool.tile([128, KC * HS], BF16)
            for k in range(KC):
                nc.sync.dma_start(out=wd_sb[:, HS * k : HS * (k + 1)],
                                  in_=Wd_s[128 * k : 128 * (k + 1), :])
            bd_sb = wpool.tile([HS, 1], F32)
            nc.sync.dma_start(out=bd_sb[:], in_=bd_s[:])

            for m in range(NROW):
                dp = dpool.tile([HS, 512], F32, tag="dp")
                for k in range(KC):
                    rh = dnpool.tile([128, 512], BF16, tag="rh")
                    nc.sync.dma_start(
                        out=rh[:],
                        in_=hsT_dram.ap()[m * TPG : (m + 1) * TPG, :, 32 * k : 32 * k + 32]
                        .rearrange("t p b -> p t b"),
                    )
                    nc.tensor.matmul(
                        dp[:],
                        lhsT=wd_sb[:, HS * k : HS * (k + 1)],
                        rhs=rh[:],
                        start=(k == 0), stop=(k == KC - 1),
                    )
                xo = dnpool.tile([HS, 512], F32, tag="xo")
                nc.scalar.activation(xo[:], dp[:], AF.Identity, bias=bd_sb[:])
                nc.sync.dma_start(out=xd_out[m], in_=xo[:])

    nc.compile()
    return nc


def make_core_inputs(c0, h0, inputs, Wi, Wh, b, Wd, bd):
    xT = np.ascontiguousarray(np.transpose(inputs, (2, 1, 0))).astype(np.float32)
    h0T_full = np.ascontiguousarray(
        np.concatenate([h0[:, 128 * j : 128 * (j + 1)].T for j in range(NC)], axis=1)
    ).astype(ml_dtypes.bfloat16)
    ident = np.eye(32, dtype=np.float32)
    ones1 = np.ones((1, 128), dtype=np.float32)

    in_maps = []
    for k in range(NC):
        sl = slice(128 * k, 128 * (k + 1))
        cols = np.concatenate([
            np.arange(2 * H, 3 * H)[sl],      # g
            np.arange(1 * H, 2 * H)[sl],      # f
            np.arange(0 * H, 1 * H)[sl],      # i
            np.arange(3 * H, 4 * H)[sl],      # o
        ])
        in_maps.append({
            "xT": xT,
            "Wi_s": np.ascontiguousarray(Wi[:, cols]).astype(np.float32),
            "Wh_s": np.ascontiguousarray(Wh[:, cols]).astype(ml_dtypes.bfloat16),
            "b_s": np.ascontiguousarray(b[cols])[None, :].astype(np.float32),
            "Wd_s": np.ascontiguousarray(Wd[:, sl]).astype(ml_dtypes.bfloat16),
            "bd_s": np.ascontiguousarray(bd[sl])[:, None].astype(np.float32),
            "c0_s": np.ascontiguousarray(c0[:, sl]).astype(np.float32),
            "h0T": h0T_full,
            "ident": ident,
            "ones1": ones1,
        })
    return in_maps


def assemble_outputs(results):
    c = np.concatenate([results[k]["c_out"] for k in range(NC)], axis=1)
    h = np.concatenate([results[k]["h_out"] for k in range(NC)], axis=1)
    x = np.empty((B, T, V), dtype=np.float32)
    for k in range(NC):
        xd = results[k]["xd_out"]                     # [NROW, 128, 512]
        arr = xd.reshape(NROW, 128, TPG, B)           # [m, v, t_local, b]
        arr = np.transpose(arr, (3, 0, 2, 1)).reshape(B, T, 128)
        x[:, :, 128 * k : 128 * (k + 1)] = arr
    return c, h, x


_NC_CACHE = {}


def _get_nc():
    if "nc" not in _NC_CACHE:
        _NC_CACHE["nc"] = build_kernel()
    return _NC_CACHE["nc"]


def kernel(c0, h0, inputs, Wi, Wh, b, Wd, bd):
    from concourse.bass_utils import run_bass_kernel_spmd

    nc = _get_nc()
    in_maps = make_core_inputs(
        np.asarray(c0), np.asarray(h0), np.asarray(inputs), np.asarray(Wi),
        np.asarray(Wh), np.asarray(b), np.asarray(Wd), np.asarray(bd),
    )
    res = run_bass_kernel_spmd(nc, in_maps, list(range(NC)))
    return assemble_outputs(res.results)


# revision 6
# speedup vs baseline: 1.1117x; 1.1117x over previous
"""nn_Decoder — LSTM (B=32, T=256, V=H=1024) + dense head on 8 TRN2 NeuronCores.

Strategy: 8-way model parallelism over the 4H gate dimension.
  - Each core owns a 128-wide slice of the hidden state and the matching
    512 columns (4 gates x 128) of Wi/Wh, gate-permuted to [i|f|o|g].
  - A = x @ Wi + b is precomputed in [128 x 512] tiles (4 timesteps per tile)
    on the tensor engine, interleaved with the recurrence, staged per step and
    injected into the z PSUM accumulation via an identity matmul.
  - Per step: z = A_t + h @ Wh_slice accumulates in PSUM (fp32r matmuls),
    sigmoid/tanh on ScalarE, state update on VectorE, h-slice transposed and
    AllGathered so every core has the full h^T for the next step.
  - Gathered h^T for every step is stored to DRAM; the dense head
    x = hs @ Wd + bd runs V-sharded (each core computes a 128-wide vocab
    slice for all (b, t) rows as x^T tiles).
Host glue shards/permutes inputs per core and reassembles (c, h, x).
"""

import numpy as np
import ml_dtypes
import concourse.bass as bass
import concourse.bacc as bacc
import concourse.mybir as mybir
from concourse import tile

F32 = mybir.dt.float32
F32R = mybir.dt.float32r
BF16 = mybir.dt.bfloat16
AF = mybir.ActivationFunctionType

B, V, H, T = 32, 1024, 1024, 256
NC = 8
SH = 4 * H // NC          # 512 per-core z-slice
HS = H // NC              # 128 per-core h-slice
KC = 8                    # 128-wide K chunks in V / H
NROW = B * T // 512       # dense row groups
TPG = 512 // B            # timesteps per dense row group


N_WARM = 0


def build_kernel(a_ahead: int = 2):
    TG = T // 4
    nc = bacc.Bacc("TRN2", target_bir_lowering=False, debug=False, num_devices=NC)

    xT = nc.dram_tensor("xT", [V, T, B], F32R, kind="ExternalInput")
    Wi_s = nc.dram_tensor("Wi_s", [V, SH], F32R, kind="ExternalInput")
    Wh_s = nc.dram_tensor("Wh_s", [H, SH], BF16, kind="ExternalInput")
    b_s = nc.dram_tensor("b_s", [1, SH], F32R, kind="ExternalInput")
    Wd_s = nc.dram_tensor("Wd_s", [H, HS], BF16, kind="ExternalInput")
    bd_s = nc.dram_tensor("bd_s", [HS, 1], F32, kind="ExternalInput")
    c0_s = nc.dram_tensor("c0_s", [B, HS], F32, kind="ExternalInput")
    h0T = nc.dram_tensor("h0T", [128, NC * B], BF16, kind="ExternalInput")
    ident = nc.dram_tensor("ident", [32, 32], F32R, kind="ExternalInput")
    ones1 = nc.dram_tensor("ones1", [1, 128], F32R, kind="ExternalInput")

    c_out = nc.dram_tensor("c_out", [B, HS], F32, kind="ExternalOutput")
    h_out = nc.dram_tensor("h_out", [B, HS], F32, kind="ExternalOutput")
    xd_out = nc.dram_tensor("xd_out", [NROW, HS, 512], F32, kind="ExternalOutput")

    cc_in = [nc.dram_tensor(f"cc_in{p}", [128, B], BF16) for p in range(2)]
    cc_out = [
        nc.dram_tensor(f"cc_out{p}", [NC * 128, B], BF16, addr_space="Shared")
        for p in range(2)
    ]
    hsT_dram = nc.dram_tensor("hsT_dram", [T, 128, NC * B], BF16)

    with tile.TileContext(nc) as tc:
        with (
            tc.tile_pool(name="weights", bufs=1) as wpool,
            tc.tile_pool(name="state", bufs=1) as spool,
            tc.tile_pool(name="hbuf", bufs=3) as hpool,
            tc.tile_pool(name="xt", bufs=2 * KC + 2) as xtpool,
            tc.tile_pool(name="astash", bufs=3) as aspool,
            tc.tile_pool(name="astage", bufs=5) as astpool,
            tc.tile_pool(name="gates", bufs=2) as gpool,
            tc.tile_pool(name="tmp", bufs=2) as tpool,
            tc.tile_pool(name="hsnd", bufs=2) as hsndpool,
            tc.tile_pool(name="zpsum", bufs=2, space="PSUM") as zpool,
            tc.tile_pool(name="apsum", bufs=2, space="PSUM") as apool,
            tc.tile_pool(name="dpsum", bufs=2, space="PSUM") as dpool,
            tc.tile_pool(name="dense", bufs=2) as dnpool,
        ):
            wi_sb = wpool.tile([128, KC * SH], F32R)
            wh_sb = wpool.tile([128, KC * SH], BF16)
            for k in range(KC):
                nc.sync.dma_start(out=wi_sb[:, SH * k : SH * (k + 1)],
                                  in_=Wi_s[128 * k : 128 * (k + 1), :])
                nc.sync.dma_start(out=wh_sb[:, SH * k : SH * (k + 1)],
                                  in_=Wh_s[128 * k : 128 * (k + 1), :])
            b_sb = wpool.tile([1, SH], F32R)
            nc.sync.dma_start(out=b_sb[:], in_=b_s[:])
            id_sb = wpool.tile([32, 32], F32R)
            nc.sync.dma_start(out=id_sb[:], in_=ident[:])
            on_sb = wpool.tile([1, 128], F32R)
            nc.sync.dma_start(out=on_sb[:], in_=ones1[:])

            c_sb = spool.tile([B, HS], F32)
            nc.sync.dma_start(out=c_sb[:], in_=c0_s[:])
            hbuf = hpool.tile([128, NC * B], BF16)
            nc.sync.dma_start(out=hbuf[:], in_=h0T[:])

            def emit_a_group(jg):
                ap = apool.tile([128, SH], F32)
                nc.tensor.matmul(ap[:], lhsT=on_sb[:], rhs=b_sb[:],
                                 start=True, stop=False)
                for k in range(KC):
                    xt = xtpool.tile([128, 128], F32R, tag="xt")
                    nc.sync.dma_start(
                        out=xt[:],
                        in_=xT[128 * k : 128 * (k + 1), 4 * jg : 4 * jg + 4, :],
                    )
                    nc.tensor.matmul(ap[:], lhsT=xt[:],
                                     rhs=wi_sb[:, SH * k : SH * (k + 1)],
                                     start=False, stop=(k == KC - 1))
                st = aspool.tile([128, SH], F32R, tag="astash")
                nc.vector.tensor_copy(st[:], ap[:])
                return st

            stash = {}
            stage = {}

            def emit_a_stage(t, st):
                sg = astpool.tile([32, SH], F32R, tag="astage")
                ts = t % 4
                nc.sync.dma_start(out=sg[:], in_=st[32 * ts : 32 * ts + 32, :])
                return sg

            for jg in range(min(a_ahead + 1, TG)):
                stash[jg] = emit_a_group(jg)
            for t in range(min(3, T)):
                stage[t] = emit_a_stage(t, stash[0])

            for t in range(T):
                bi = t % 2
                jg = t // 4
                if t % 4 == 0:
                    ng = jg + a_ahead + 1
                    if ng < TG and ng not in stash:
                        stash[ng] = emit_a_group(ng)
                        stash.pop(ng - a_ahead - 2, None)
                for ts in (t + 2, t + 3):
                    if ts < T and ts not in stage and (ts // 4) in stash:
                        stage[ts] = emit_a_stage(ts, stash[ts // 4])

                # half 1: gates g, f   |   half 2: gates i, o
                z1 = zpool.tile([B, SH // 2], F32, tag="z1")
                z2 = zpool.tile([B, SH // 2], F32, tag="z2")
                nc.tensor.matmul(z1[:], lhsT=id_sb[:], rhs=stage[t][:, 0:256],
                                 start=True, stop=False)
                for j in range(KC):
                    nc.tensor.matmul(
                        z1[:],
                        lhsT=hbuf[:, 32 * j : 32 * j + 32],
                        rhs=wh_sb[:, SH * j : SH * j + 256],
                        start=False, stop=(j == KC - 1),
                    )
                nc.tensor.matmul(z2[:], lhsT=id_sb[:], rhs=stage[t][:, 256:512],
                                 start=True, stop=False)
                for j in range(KC):
                    nc.tensor.matmul(
                        z2[:],
                        lhsT=hbuf[:, 32 * j : 32 * j + 32],
                        rhs=wh_sb[:, SH * j + 256 : SH * (j + 1)],
                        start=False, stop=(j == KC - 1),
                    )
                stage.pop(t, None)

                g_t = gpool.tile([B, 128], F32, tag="g")
                nc.scalar.activation(g_t[:], z1[:, 0:128], AF.Tanh)
                sf = gpool.tile([B, 128], F32, tag="sf")
                nc.scalar.activation(sf[:], z1[:, 128:256], AF.Sigmoid)
                sio = gpool.tile([B, 256], F32, tag="sio")
                nc.scalar.activation(sio[:], z2[:], AF.Sigmoid)

                fc = tpool.tile([B, 128], F32, tag="fc")
                nc.vector.tensor_mul(fc[:], sf[:], c_sb[:])
                ig = tpool.tile([B, 128], F32, tag="ig")
                nc.vector.tensor_mul(ig[:], sio[:, 0:128], g_t[:])
                nc.vector.tensor_add(c_sb[:], fc[:], ig[:])
                tc_t = tpool.tile([B, 128], F32, tag="tc")
                nc.scalar.activation(tc_t[:], c_sb[:], AF.Tanh)
                h_sb = tpool.tile([B, 128], BF16, tag="h")
                nc.vector.tensor_mul(h_sb[:], sio[:, 128:256], tc_t[:])

                if t == T - 1:
                    nc.sync.dma_start(out=c_out[:], in_=c_sb[:])
                    hf = tpool.tile([B, 128], F32, tag="hf")
                    nc.vector.tensor_mul(hf[:], sio[:, 128:256], tc_t[:])
                    nc.sync.dma_start(out=h_out[:], in_=hf[:])

                hT = hsndpool.tile([128, B], BF16, tag="hT")
                for q in range(4):
                    nc.vector.transpose(hT[32 * q : 32 * q + 32, :],
                                        h_sb[:, 32 * q : 32 * q + 32])

                snd_dma = nc.sync.dma_start(out=cc_in[bi][:], in_=hT[:])
                nc.gpsimd.collective_compute(
                    "AllGather",
                    mybir.AluOpType.bypass,
                    ins=[cc_in[bi][:]],
                    outs=[cc_out[bi][:]],
                    replica_groups=[list(range(NC))],
                )
                hbuf = hpool.tile([128, NC * B], BF16, tag="hbuf")
                nc.sync.dma_start(
                    out=hbuf[:],
                    in_=cc_out[bi].ap().rearrange("(j p) b -> p j b", p=128),
                )
                nc.sync.dma_start(out=hsT_dram[t], in_=hbuf[:])

            # dense head: x^T v-slice
            wd_sb = wpool.tile([128, KC * HS], BF16)
            for k in range(KC):
                nc.sync.dma_start(out=wd_sb[:, HS * k : HS * (k + 1)],
                                  in_=Wd_s[128 * k : 128 * (k + 1), :])
            bd_sb = wpool.tile([HS, 1], F32)
            nc.sync.dma_start(out=bd_sb[:], in_=bd_s[:])

            for m in range(NROW):
                dp = dpool.tile([HS, 512], F32, tag="dp")
                for k in range(KC):
                    rh = dnpool.tile([128, 512], BF16, tag="rh")
                    nc.sync.dma_start(
                        out=rh[:],
                        in_=hsT_dram.ap()[m * TPG : (m + 1) * TPG, :, 32 * k : 32 * k + 32]
                        .rearrange("t p b -> p t b"),
                    )
                    nc.tensor.matmul(
                        dp[:],
                        lhsT=wd_sb[:, HS * k : HS * (k + 1)],
                        rhs=rh[:],
                        start=(k == 0), stop=(k == KC - 1),
                    )
                xo = dnpool.tile([HS, 512], F32, tag="xo")
                nc.scalar.activation(xo[:], dp[:], AF.Identity, bias=bd_sb[:])
                nc.sync.dma_start(out=xd_out[m], in_=xo[:])

    nc.compile()
    return nc


def make_core_inputs(c0, h0, inputs, Wi, Wh, b, Wd, bd):
    xT = np.ascontiguousarray(np.transpose(inputs, (2, 1, 0))).astype(np.float32)
    h0T_full = np.ascontiguousarray(
        np.concatenate([h0[:, 128 * j : 128 * (j + 1)].T for j in range(NC)], axis=1)
    ).astype(ml_dtypes.bfloat16)
    ident = np.eye(32, dtype=np.float32)
    ones1 = np.ones((1, 128), dtype=np.float32)

    in_maps = []
    for k in range(NC):
        sl = slice(128 * k, 128 * (k + 1))
        cols = np.concatenate([
            np.arange(2 * H, 3 * H)[sl],      # g
            np.arange(1 * H, 2 * H)[sl],      # f
            np.arange(0 * H, 1 * H)[sl],      # i
            np.arange(3 * H, 4 * H)[sl],      # o
        ])
        in_maps.append({
            "xT": xT,
            "Wi_s": np.ascontiguousarray(Wi[:, cols]).astype(np.float32),
            "Wh_s": np.ascontiguousarray(Wh[:, cols]).astype(ml_dtypes.bfloat16),
            "b_s": np.ascontiguousarray(b[cols])[None, :].astype(np.float32),
            "Wd_s": np.ascontiguousarray(Wd[:, sl]).astype(ml_dtypes.bfloat16),
            "bd_s": np.ascontiguousarray(bd[sl])[:, None].astype(np.float32),
            "c0_s": np.ascontiguousarray(c0[:, sl]).astype(np.float32),
            "h0T": h0T_full,
            "ident": ident,
            "ones1": ones1,
        })
    return in_maps


def assemble_outputs(results):
    c = np.concatenate([results[k]["c_out"] for k in range(NC)], axis=1)
    h = np.concatenate([results[k]["h_out"] for k in range(NC)], axis=1)
    x = np.empty((B, T, V), dtype=np.float32)
    for k in range(NC):
        xd = results[k]["xd_out"]                     # [NROW, 128, 512]
        arr = xd.reshape(NROW, 128, TPG, B)           # [m, v, t_local, b]
        arr = np.transpose(arr, (3, 0, 2, 1)).reshape(B, T, 128)
        x[:, :, 128 * k : 128 * (k + 1)] = arr
    return c, h, x


_NC_CACHE = {}


def _get_nc():
    if "nc" not in _NC_CACHE:
        _NC_CACHE["nc"] = build_kernel()
    return _NC_CACHE["nc"]


def kernel(c0, h0, inputs, Wi, Wh, b, Wd, bd):
    from concourse.bass_utils import run_bass_kernel_spmd

    nc = _get_nc()
    in_maps = make_core_inputs(
        np.asarray(c0), np.asarray(h0), np.asarray(inputs), np.asarray(Wi),
        np.asarray(Wh), np.asarray(b), np.asarray(Wd), np.asarray(bd),
    )
    res = run_bass_kernel_spmd(nc, in_maps, list(range(NC)))
    return assemble_outputs(res.results)
